# revision 38
# baseline (speedup 1.0000x reference)
"""Trainium2 Bass kernel for nn_BaseAttention (B=2,S=2048,D=1024,H=16,K=64).

Sharding v2: 8 cores = (batch b in {0,1}) x (head-group g in {0..3}, 4 heads).
Each core computes Q/K/V projections for its 4 heads over the FULL sequence,
attention for its 4 heads (all 2048 q rows), then an AllToAll within each
4-core batch group redistributes context so core (b,g) holds ALL 16 heads for
q-block g (512 rows). Output projection + residual + LayerNorm on that block.
No redundant compute; the only collective is a 1MB AllToAll of bf16 context.

Per-core engine plan:
  PE    : all matmuls in bf16 (1 cycle/row vs 1.5 for f32r).
          - scores:   per head-pair, row-tiled (heads at partitions 0-63 /
            64-127 -> tile_position rows 0/64) so two 64-contraction matmuls
            run concurrently in the 128x128 array.
          - context:  per head-pair, col-tiled (outputs at PSUM partitions
            0-63 / 64-127) -> concurrent.
          - denoms:   sum_s exp via ones-column stationary [128,1], col-tiled
            4 ways at PSUM partitions 0/32/64/96.
          - V natural layout obtained by PE-transposing V^T tiles.
  ACT   : exact exp (table) on a share of score tiles; LN square/sqrt.
  DVE   : Schraudolph bf16 exp (bitcast int16(x*184.665+B)) on the rest;
          PSUM->SBUF copies with bias; softmax normalize; LN elementwise.
  GPSIMD: triggers the AllToAll.
Scores are computed pre-scaled: wq is folded with (1/sqrt(64))*184.665 so the
DVE exp is a single tensor_scalar add, and ACT exp uses scale=1/184.665.
"""

import sys
import numpy as np

B, S, D, H, KD = 2, 2048, 1024, 16, 64
P = 128
GH = 4                 # heads per core
GHK = GH * KD          # 256
SB = S // 4            # 512 output rows per core
NQC = 4                # q chunks of 512
QW = 512
NSC = S // P           # 16 key chunks
DC = D // P            # 8 contraction chunks
NT = GHK // P          # 2 tiles (= head pairs) per core
HC = H * KD // P       # 8 hk tiles globally

EXPA = 184.6649652337873        # 2^7 / ln2
EXPB = 16250.65                 # Schraudolph bias (bf16), tuned numerically
ACT_FRAC_NUM, ACT_FRAC_DEN = 9, 16   # fraction of exp tiles on ACT engine

if "/opt/trn_rl_repo" not in sys.path:
    sys.path.insert(0, "/opt/trn_rl_repo")

_cache = {}


def _build():
    import concourse.bass as bass
    import concourse.mybir as mybir
    from concourse.tile import TileContext

    dt = mybir.dt
    f32, bf16, i16 = dt.float32, dt.bfloat16, dt.int16
    AF = mybir.ActivationFunctionType
    OP = mybir.AluOpType
    AX = mybir.AxisListType.X

    nc = bass.Bass()
    xT = nc.declare_dram_parameter("xT", [D, S], bf16, isOutput=False)
    wq = nc.declare_dram_parameter("wq", [D, GHK], bf16, isOutput=False)
    wk = nc.declare_dram_parameter("wk", [D, GHK], bf16, isOutput=False)
    wv = nc.declare_dram_parameter("wv", [D, GHK], bf16, isOutput=False)
    wo = nc.declare_dram_parameter("wo", [H * KD, D], bf16, isOutput=False)
    bqp = nc.declare_dram_parameter("bqT", [P, NT], f32, isOutput=False)
    bkp = nc.declare_dram_parameter("bkT", [P, NT], f32, isOutput=False)
    bvp = nc.declare_dram_parameter("bvT", [P, NT], f32, isOutput=False)
    xqb = nc.declare_dram_parameter("xqb", [SB, D], f32, isOutput=False)
    gmp = nc.declare_dram_parameter("gamma_row", [1, D], bf16, isOutput=False)
    btp = nc.declare_dram_parameter("beta_row", [1, D], f32, isOutput=False)
    idp = nc.declare_dram_parameter("ident", [P, P], bf16, isOutput=False)
    slp = nc.declare_dram_parameter("selm", [P, 2 * P], bf16, isOutput=False)
    out = nc.declare_dram_parameter("out", [SB, D], f32, isOutput=True)

    with TileContext(nc) as tc:
        with tc.tile_pool(name="const", bufs=1) as cpool, \
             tc.tile_pool(name="qk", bufs=NT) as qkp, \
             tc.tile_pool(name="va", bufs=NSC) as vap, \
             tc.tile_pool(name="wo", bufs=HC) as wop, \
             tc.tile_pool(name="cn", bufs=2 * NQC) as cnp, \
             tc.tile_pool(name="dram", bufs=2, space="DRAM") as drp:

            ones_bf = cpool.tile([P, 1], bf16, tag="ones")
            nc.vector.memset(ones_bf[:], 1.0)
            eps_t = cpool.tile([P, 1], f32, tag="eps")
            nc.vector.memset(eps_t[:], 1e-6)
            ident = cpool.tile([P, P], bf16, tag="ident")
            selm = cpool.tile([P, 2 * P], bf16, tag="selm")
            bq_sb = cpool.tile([P, NT], f32, tag="bq")
            bk_sb = cpool.tile([P, NT], f32, tag="bk")
            bv_sb = cpool.tile([P, NT], f32, tag="bv")
            gmB = cpool.tile([P, D], bf16, tag="gmB")
            btB = cpool.tile([P, D], f32, tag="btB")

            # AllToAll over all 8 cores: block j (rows 256j..256j+255) goes to
            # core j and holds this core's 4 heads (2 pair-tiles) for q rows
            # [256j, 256j+256) of this core's batch. Core j outputs those q
            # rows for BOTH batches, so every block is meaningful and all
            # offsets are batch-independent (pure SPMD). One collective: its
            # ~40us cost is rendezvous latency, independent of payload size.
            a2a_in = drp.tile([8 * 2 * P, 2 * P], bf16, tag="a2a_in")
            a2a_out = drp.tile([8 * 2 * P, 2 * P], bf16, tag="a2a_out")

            # ---- projections: Q^T, K^T, V^T [GHK, S], all bf16 ----
            qt_sb, kt_sb, vt_sb, va_sb = [], [], [], []
            vt_scope = tc.tile_pool(name="vt", bufs=NT)
            vtp = vt_scope.__enter__()
            with tc.tile_pool(name="xT", bufs=DC) as xtp, \
                 tc.tile_pool(name="w3", bufs=DC) as w3p, \
                 tc.tile_pool(name="pp", bufs=2, space="PSUM") as pp:
                xt_sb, wq_sb, wk_sb, wv_sb = [], [], [], []
                for dc in range(DC):
                    t = w3p.tile([P, GHK], bf16, tag="wq")
                    nc.sync.dma_start(out=t[:], in_=wq[dc * P:(dc + 1) * P, :])
                    wq_sb.append(t)
                # xT loaded in [P, QW] chunks, m-major, so the chunks feeding
                # the first accumulation group land on many DMA queues in
                # parallel and the first matmul can start ~4x sooner.
                for dc in range(DC):
                    xt_sb.append(xtp.tile([P, S], bf16, tag="xt",
                                          name=f"xt{dc}"))
                for m in range(S // QW):
                    for dc in range(DC):
                        nc.sync.dma_start(
                            out=xt_sb[dc][:, m * QW:(m + 1) * QW],
                            in_=xT[dc * P:(dc + 1) * P, m * QW:(m + 1) * QW])
                for dc in range(DC):
                    t = w3p.tile([P, GHK], bf16, tag="wk")
                    nc.sync.dma_start(out=t[:], in_=wk[dc * P:(dc + 1) * P, :])
                    wk_sb.append(t)
                for dc in range(DC):
                    t = w3p.tile([P, GHK], bf16, tag="wv")
                    nc.sync.dma_start(out=t[:], in_=wv[dc * P:(dc + 1) * P, :])
                    wv_sb.append(t)
                # constants loaded after the critical-path weight/x loads so
                # they don't occupy the first DMA queue slots
                nc.sync.dma_start(out=bq_sb[:], in_=bqp[:])
                nc.sync.dma_start(out=bk_sb[:], in_=bkp[:])
                nc.sync.dma_start(out=bv_sb[:], in_=bvp[:])
                nc.sync.dma_start(out=ident[:], in_=idp[:])
                nc.sync.dma_start(out=selm[:], in_=slp[:])
                nc.sync.dma_start(out=gmB[:], in_=gmp[:].to_broadcast((P, D)))
                nc.sync.dma_start(out=btB[:], in_=btp[:].to_broadcast((P, D)))

                def project(w_sb, b_sb, out_list, pool, tag):
                    for t in range(NT):
                        pq = pp.tile([P, S], f32, tag="pp")
                        for m in range(S // QW):
                            for dc in range(DC):
                                nc.tensor.matmul(
                                    pq[:, m * QW:(m + 1) * QW],
                                    w_sb[dc][:, t * P:(t + 1) * P],
                                    xt_sb[dc][:, m * QW:(m + 1) * QW],
                                    start=(dc == 0), stop=(dc == DC - 1))
                        ot = pool.tile([P, S], bf16, tag=tag)
                        nc.vector.tensor_scalar(ot[:], pq[:], b_sb[:, t:t + 1],
                                                None, OP.add)
                        out_list.append(ot)

                project(wq_sb, bq_sb, qt_sb, qkp, "qt")
                project(wk_sb, bk_sb, kt_sb, qkp, "kt")
                project(wv_sb, bv_sb, vt_sb, vtp, "vt")
            # ---- V to natural layout [s, hk] via PE transpose ----
            with tc.tile_pool(name="tp", bufs=4, space="PSUM") as tpp:
                for sc in range(NSC):
                    pvt = tpp.tile([P, GHK], bf16, tag="tp")
                    with nc.allow_low_precision(
                            reason="pure transpose, no accumulation"):
                        for t in range(NT):
                            nc.tensor.transpose(
                                pvt[:, t * P:(t + 1) * P],
                                vt_sb[t][:, sc * P:(sc + 1) * P],
                                ident[:])
                    va = vap.tile([P, GHK], bf16, tag="va")
                    nc.vector.tensor_copy(va[:], pvt[:])
                    va_sb.append(va)
            vt_scope.__exit__(None, None, None)

            # wo / xq loads (needed late; queue after projection loads)
            wo_sb = []
            for hc in range(HC):
                t = wop.tile([P, D], bf16, tag="wo")
                nc.sync.dma_start(out=t[:], in_=wo[hc * P:(hc + 1) * P, :])
                wo_sb.append(t)
            xq_sb = []
            with tc.tile_pool(name="xq", bufs=4) as xqp:
                for qs in range(SB // P):
                    t = xqp.tile([P, D], f32, tag="xq")
                    nc.sync.dma_start(out=t[:], in_=xqb[qs * P:(qs + 1) * P, :])
                    xq_sb.append(t)

                # ---- attention, qc-major: per sc both pairs' exps run on ----
                # ---- both exp engines concurrently (strict alternation) ----
                QA = QW
                with tc.tile_pool(name="ps", bufs=2, space="PSUM") as psp, \
                     tc.tile_pool(name="pc", bufs=3, space="PSUM") as pcp, \
                     tc.tile_pool(name="dn", bufs=1, space="PSUM") as dnp, \
                     tc.tile_pool(name="et", bufs=8) as etp, \
                     tc.tile_pool(name="dsb", bufs=2) as dsbp, \
                     tc.tile_pool(name="rb", bufs=3) as rbp:
                    for qc in range(S // QA):
                        pc = [pcp.tile([P, QA], f32, tag="pc",
                                       name=f"pc{qc}_{i}") for i in range(NT)]
                        pdn = dnp.tile([P, QA], f32, tag="dn",
                                       name=f"pdn{qc}")
                        nc.vector.memset(pdn[:], 0.0)
                        ets = [None] * NSC

                        def emit_scores(sc):
                            pair_ets = []
                            for pr in range(NT):
                                ps = psp.tile([P, 2 * QA], f32, tag="ps")
                                for j in range(2):
                                    nc.tensor.matmul(
                                        ps[:, j * QA:(j + 1) * QA],
                                        kt_sb[pr][j * KD:(j + 1) * KD,
                                                  sc * P:(sc + 1) * P],
                                        qt_sb[pr][j * KD:(j + 1) * KD,
                                                  qc * QA:(qc + 1) * QA],
                                        start=True, stop=True)
                                # one full-width exp per pair tile; pairs
                                # alternate strictly between ACT (exact exp)
                                # and DVE (Schraudolph bitcast exp) so both
                                # engines run every sc step
                                if (pr + sc) % 2 == 0:
                                    et = etp.tile([P, 2 * QA], bf16, tag="et")
                                    nc.scalar.activation(et[:], ps[:], AF.Exp,
                                                         scale=1.0 / EXPA)
                                    etv = et[:]
                                else:
                                    et = etp.tile([P, 2 * QA], i16, tag="et")
                                    nc.vector.tensor_scalar(
                                        et[:], ps[:], EXPB, 0.0,
                                        OP.add, OP.max)
                                    etv = et[:].bitcast(bf16)
                                pair_ets.append(etv)
                            ets[sc] = pair_ets

                        def emit_ctx(sc):
                            for pr in range(NT):
                                etv = ets[sc][pr]
                                for j in range(2):
                                    nc.tensor.matmul(
                                        pc[pr][j * KD:(j + 1) * KD, :],
                                        va_sb[sc][:, (2 * pr + j) * KD:
                                                  (2 * pr + j + 1) * KD],
                                        etv[:, j * QA:(j + 1) * QA],
                                        start=(sc == 0), stop=(sc == NSC - 1))
                            for pr in range(NT):
                                etv = ets[sc][pr]
                                for j in range(2):
                                    r = 32 * (2 * pr + j)
                                    nc.tensor.matmul(
                                        pdn[r:r + 1, :],
                                        ones_bf[:],
                                        etv[:, j * QA:(j + 1) * QA],
                                        start=(sc == 0), stop=(sc == NSC - 1),
                                        tile_position=(0, r))
                            ets[sc] = None

                        for sc in range(NSC):
                            emit_scores(sc)
                            if sc > 0:
                                emit_ctx(sc - 1)
                        emit_ctx(NSC - 1)

                        # normalize: cn = pc * (1/denom bcast); the
                        # reciprocal runs as exp(-ln(x)) on ACT (the DVE
                        # reciprocal instruction is ~4 cycles/element)
                        dn_t = dsbp.tile([P, QA], bf16, tag="dsb")
                        nc.scalar.activation(dn_t[:], pdn[:], AF.Copy)
                        for pr in range(NT):
                            prb = dnp.tile([P, QA], f32, tag="dn",
                                           name=f"prb{qc}_{pr}")
                            nc.tensor.matmul(prb[:],
                                             selm[:, pr * P:(pr + 1) * P],
                                             dn_t[:], start=True, stop=True)
                            lnt = rbp.tile([P, QA], f32, tag="lnt")
                            nc.scalar.activation(lnt[:], prb[:], AF.Ln)
                            rb_t = rbp.tile([P, QA], bf16, tag="rb")
                            nc.scalar.activation(rb_t[:], lnt[:], AF.Exp,
                                                 scale=-1.0)
                            cn = cnp.tile([P, QA], bf16, tag="cn")
                            nc.vector.tensor_tensor(cn[:], pc[pr][:], rb_t[:],
                                                    OP.mult)
                            for h in range(2):
                                blk = 2 * qc + h
                                nc.sync.dma_start(
                                    out=a2a_in[blk * 2 * P + pr * P:
                                               blk * 2 * P + (pr + 1) * P, :],
                                    in_=cn[:, h * 2 * P:(h + 1) * 2 * P])

                # ---- AllToAll across all 8 cores ----
                nc.gpsimd.collective_compute(
                    "AllToAll", mybir.AluOpType.bypass,
                    replica_groups=[[0, 1, 2, 3, 4, 5, 6, 7]],
                    ins=[a2a_in[:].opt()],
                    outs=[a2a_out[:].opt()])

                # ---- output projection + residual + LayerNorm ----
                with tc.tile_pool(name="ctxa", bufs=2 * HC) as ctxp, \
                     tc.tile_pool(name="op", bufs=2, space="PSUM") as opp, \
                     tc.tile_pool(name="ln", bufs=2) as lnp, \
                     tc.tile_pool(name="st", bufs=4) as stp:
                    ctx_all = {}
                    for b_ in range(2):
                        for hc in range(HC):
                            t = ctxp.tile([P, 2 * P], bf16, tag="ctxa")
                            src = (4 * b_ + hc // 2) * 2 * P + (hc % 2) * P
                            nc.sync.dma_start(
                                out=t[:], in_=a2a_out[src:src + P, :])
                            ctx_all[(b_, hc)] = t
                    for qs in range(SB // P):
                        b_, q2 = qs // 2, qs % 2
                        po = opp.tile([P, D], f32, tag="op")
                        for d5 in range(D // QW):
                            for hc in range(HC):
                                nc.tensor.matmul(
                                    po[:, d5 * QW:(d5 + 1) * QW],
                                    ctx_all[(b_, hc)][:, q2 * P:(q2 + 1) * P],
                                    wo_sb[hc][:, d5 * QW:(d5 + 1) * QW],
                                    start=(hc == 0), stop=(hc == HC - 1))
                        y = lnp.tile([P, D], bf16, tag="y")
                        nc.vector.tensor_tensor(y[:], po[:], xq_sb[qs][:],
                                                OP.add)
                        sum_t = stp.tile([P, 1], f32, tag="sum")
                        nc.vector.reduce_sum(out=sum_t[:], in_=y[:], axis=AX)
                        mean_t = stp.tile([P, 1], f32, tag="mean")
                        nc.vector.tensor_scalar_mul(mean_t[:], sum_t[:],
                                                    1.0 / D)
                        cent = lnp.tile([P, D], bf16, tag="cent")
                        nc.vector.tensor_scalar(cent[:], y[:], mean_t[:],
                                                None, OP.subtract)
                        sq = lnp.tile([P, D], bf16, tag="sq")
                        vs = stp.tile([P, 1], f32, tag="vs")
                        nc.scalar.activation(sq[:], cent[:], AF.Square,
                                             accum_out=vs[:])
                        std = stp.tile([P, 1], f32, tag="std")
                        nc.scalar.activation(std[:], vs[:], AF.Sqrt,
                                             bias=eps_t[:], scale=1.0 / D)
                        rstd = stp.tile([P, 1], f32, tag="rstd")
                        nc.vector.reciprocal(rstd[:], std[:])
                        z = lnp.tile([P, D], bf16, tag="z")
                        nc.vector.tensor_scalar_mul(z[:], cent[:], rstd[:])
                        zg = lnp.tile([P, D], bf16, tag="zg")
                        nc.vector.tensor_tensor(zg[:], z[:], gmB[:], OP.mult)
                        ot = lnp.tile([P, D], f32, tag="ot")
                        nc.vector.tensor_tensor(ot[:], zg[:], btB[:], OP.add)
                        nc.sync.dma_start(out=out[qs * P:(qs + 1) * P, :],
                                          in_=ot[:])

    # Post-pass: walrus's per-instruction ISA structs hold only ONE sync
    # wait for compute-engine instructions. Move excess waits onto standalone
    # EventSemaphore instructions placed just before on the same engine
    # stream (sequencer executes them in order; semantics unchanged).
    SPLIT = {"InstMatmult", "InstTensorScalarPtr", "InstTensorScalar",
             "InstTensorTensor", "InstReciprocal", "InstActivation",
             "InstTensorReduce", "InstTensorCopy", "InstMemSet",
             "InstCopy", "InstDMACopy", "InstDMATranspose", "InstDrain",
             "InstCollectiveCompute", "InstMemset"}
    evt_n = 0
    for f in nc.m.functions:
        for bb in f.blocks:
            need = any(
                type(i).__name__ in SPLIT and i.sync_info is not None
                and len(i.sync_info.on_wait) > 1 for i in bb.instructions)
            if not need:
                continue
            newl = []
            for ins in bb.instructions:
                si = ins.sync_info
                if (type(ins).__name__ in SPLIT and si is not None
                        and len(si.on_wait) > 1):
                    extra = list(si.on_wait[:-1])
                    for j in range(0, len(extra), 2):  # evt-sem holds <=2
                        evt_n += 1
                        evt = mybir.InstEventSemaphore(name=f"mmwait_{evt_n}")
                        evt.engine = ins.engine
                        evt.sync_info = mybir.SyncInfo(
                            on_wait=extra[j:j + 2], on_update=[])
                        newl.append(evt)
                    ins.sync_info = mybir.SyncInfo(
                        on_wait=[si.on_wait[-1]],
                        on_update=list(si.on_update))
                newl.append(ins)
            bb.instructions = newl
    return nc


def get_nc():
    if "nc" not in _cache:
        _cache["nc"] = _build()
    return _cache["nc"]


def make_in_maps(inputs, n_cores=8):
    """Shard full inputs into per-core input maps (host-side layout prep)."""
    from ml_dtypes import bfloat16
    f = np.float32
    HK = H * KD
    qscale = EXPA / np.sqrt(KD)
    wq_ = np.asarray(inputs["wq"], f).reshape(D, HK) * qscale
    wk_ = np.asarray(inputs["wk"], f).reshape(D, HK)
    wv_ = np.asarray(inputs["wv"], f).reshape(D, HK)
    wo_ = np.ascontiguousarray(
        np.asarray(inputs["wo"], f).reshape(HK, D)).astype(bfloat16)
    bq_ = np.asarray(inputs["bq"], f).reshape(HK) * qscale
    bk_ = np.asarray(inputs["bk"], f).reshape(HK)
    bv_ = np.asarray(inputs["bv"], f).reshape(HK)
    bo_ = np.asarray(inputs["bo"], f).reshape(D)
    gm_row = np.asarray(inputs["gamma"], f).reshape(1, D).astype(bfloat16)
    bt_row = np.asarray(inputs["beta"], f).reshape(1, D)
    ident = np.eye(P, dtype=f).astype(bfloat16)
    selm = np.zeros((P, 2 * P), f)
    for pr in range(NT):
        selm[64 * pr, pr * P: pr * P + 64] = 1.0
        selm[64 * pr + 32, pr * P + 64: pr * P + 128] = 1.0
    selm = selm.astype(bfloat16)

    xT_cache, x_cache = {}, {}
    wcache = {}
    maps = []
    for c in range(n_cores):
        b, g = c // 4, c % 4
        if b not in xT_cache:
            xb = np.asarray(inputs["x"][b], f)
            x_cache[b] = xb
            xT_cache[b] = np.ascontiguousarray(xb.T).astype(bfloat16)
        if g not in wcache:
            hk = slice(g * GHK, (g + 1) * GHK)
            wcache[g] = dict(
                wq=np.ascontiguousarray(wq_[:, hk]).astype(bfloat16),
                wk=np.ascontiguousarray(wk_[:, hk]).astype(bfloat16),
                wv=np.ascontiguousarray(wv_[:, hk]).astype(bfloat16),
                bqT=np.ascontiguousarray(bq_[hk].reshape(NT, P).T),
                bkT=np.ascontiguousarray(bk_[hk].reshape(NT, P).T),
                bvT=np.ascontiguousarray(bv_[hk].reshape(NT, P).T),
            )
        # output rows [256c, 256c+256) of BOTH batches (residual+bias input)
        xqb = np.concatenate([
            x_cache.setdefault(
                bb, np.asarray(inputs["x"][bb], f))[
                    256 * c:256 * (c + 1)] + bo_
            for bb in range(2)], axis=0)
        maps.append(dict(
            xT=xT_cache[b],
            wo=wo_,
            xqb=xqb,
            gamma_row=gm_row, beta_row=bt_row,
            ident=ident, selm=selm,
            **wcache[g],
        ))
    return maps


def assemble(res):
    outp = np.empty((B, S, D), np.float32)
    for c in range(8):
        o = res.results[c]["out"]
        for b_ in range(2):
            outp[b_, 256 * c:256 * (c + 1)] = o[256 * b_:256 * (b_ + 1)]
    return outp


def kernel(**inputs):
    from concourse.bass_utils import run_bass_kernel_spmd
    nc = get_nc()
    maps = make_in_maps(inputs)
    res = run_bass_kernel_spmd(nc, maps, list(range(8)))
    return assemble(res)


# revision 39
# speedup vs baseline: 1.0137x; 1.0137x over previous
"""Trainium2 Bass kernel for nn_BaseAttention (B=2,S=2048,D=1024,H=16,K=64).

Sharding v2: 8 cores = (batch b in {0,1}) x (head-group g in {0..3}, 4 heads).
Each core computes Q/K/V projections for its 4 heads over the FULL sequence,
attention for its 4 heads (all 2048 q rows), then an AllToAll within each
4-core batch group redistributes context so core (b,g) holds ALL 16 heads for
q-block g (512 rows). Output projection + residual + LayerNorm on that block.
No redundant compute; the only collective is a 1MB AllToAll of bf16 context.

Per-core engine plan:
  PE    : all matmuls in bf16 (1 cycle/row vs 1.5 for f32r).
          - scores:   per head-pair, row-tiled (heads at partitions 0-63 /
            64-127 -> tile_position rows 0/64) so two 64-contraction matmuls
            run concurrently in the 128x128 array.
          - context:  per head-pair, col-tiled (outputs at PSUM partitions
            0-63 / 64-127) -> concurrent.
          - denoms:   sum_s exp via ones-column stationary [128,1], col-tiled
            4 ways at PSUM partitions 0/32/64/96.
          - V natural layout obtained by PE-transposing V^T tiles.
  ACT   : exact exp (table) on a share of score tiles; LN square/sqrt.
  DVE   : Schraudolph bf16 exp (bitcast int16(x*184.665+B)) on the rest;
          PSUM->SBUF copies with bias; softmax normalize; LN elementwise.
  GPSIMD: triggers the AllToAll.
Scores are computed pre-scaled: wq is folded with (1/sqrt(64))*184.665 so the
DVE exp is a single tensor_scalar add, and ACT exp uses scale=1/184.665.
"""

import sys
import numpy as np

B, S, D, H, KD = 2, 2048, 1024, 16, 64
P = 128
GH = 4                 # heads per core
GHK = GH * KD          # 256
SB = S // 4            # 512 output rows per core
NQC = 4                # q chunks of 512
QW = 512
NSC = S // P           # 16 key chunks
DC = D // P            # 8 contraction chunks
NT = GHK // P          # 2 tiles (= head pairs) per core
HC = H * KD // P       # 8 hk tiles globally

EXPA = 184.6649652337873        # 2^7 / ln2
EXPB = 16250.65                 # Schraudolph bias (bf16), tuned numerically
ACT_FRAC_NUM, ACT_FRAC_DEN = 9, 16   # fraction of exp tiles on ACT engine

if "/opt/trn_rl_repo" not in sys.path:
    sys.path.insert(0, "/opt/trn_rl_repo")

_cache = {}


def _build():
    import concourse.bass as bass
    import concourse.mybir as mybir
    from concourse.tile import TileContext

    dt = mybir.dt
    f32, bf16, i16 = dt.float32, dt.bfloat16, dt.int16
    AF = mybir.ActivationFunctionType
    OP = mybir.AluOpType
    AX = mybir.AxisListType.X

    nc = bass.Bass()
    xT = nc.declare_dram_parameter("xT", [D, S], bf16, isOutput=False)
    wq = nc.declare_dram_parameter("wq", [D, GHK], bf16, isOutput=False)
    wk = nc.declare_dram_parameter("wk", [D, GHK], bf16, isOutput=False)
    wv = nc.declare_dram_parameter("wv", [D, GHK], bf16, isOutput=False)
    wo = nc.declare_dram_parameter("wo", [H * KD, D], bf16, isOutput=False)
    bqp = nc.declare_dram_parameter("bqT", [P, NT], f32, isOutput=False)
    bkp = nc.declare_dram_parameter("bkT", [P, NT], f32, isOutput=False)
    bvp = nc.declare_dram_parameter("bvT", [P, NT], f32, isOutput=False)
    xqb = nc.declare_dram_parameter("xqb", [SB, D], f32, isOutput=False)
    gmp = nc.declare_dram_parameter("gamma_row", [1, D], bf16, isOutput=False)
    btp = nc.declare_dram_parameter("beta_row", [1, D], f32, isOutput=False)
    idp = nc.declare_dram_parameter("ident", [P, P], bf16, isOutput=False)
    slp = nc.declare_dram_parameter("selm", [P, 2 * P], bf16, isOutput=False)
    out = nc.declare_dram_parameter("out", [SB, D], f32, isOutput=True)

    with TileContext(nc) as tc:
        with tc.tile_pool(name="const", bufs=1) as cpool, \
             tc.tile_pool(name="qk", bufs=NT) as qkp, \
             tc.tile_pool(name="va", bufs=NSC) as vap, \
             tc.tile_pool(name="wo", bufs=HC) as wop, \
             tc.tile_pool(name="cn", bufs=2 * NQC) as cnp, \
             tc.tile_pool(name="dram", bufs=2, space="DRAM") as drp:

            ones_bf = cpool.tile([P, 1], bf16, tag="ones")
            nc.vector.memset(ones_bf[:], 1.0)
            eps_t = cpool.tile([P, 1], f32, tag="eps")
            nc.vector.memset(eps_t[:], 1e-6)
            ident = cpool.tile([P, P], bf16, tag="ident")
            nc.sync.dma_start(out=ident[:], in_=idp[:])
            selm = cpool.tile([P, 2 * P], bf16, tag="selm")
            nc.sync.dma_start(out=selm[:], in_=slp[:])
            bq_sb = cpool.tile([P, NT], f32, tag="bq")
            nc.sync.dma_start(out=bq_sb[:], in_=bqp[:])
            bk_sb = cpool.tile([P, NT], f32, tag="bk")
            nc.sync.dma_start(out=bk_sb[:], in_=bkp[:])
            bv_sb = cpool.tile([P, NT], f32, tag="bv")
            nc.sync.dma_start(out=bv_sb[:], in_=bvp[:])
            gmB = cpool.tile([P, D], bf16, tag="gmB")
            nc.sync.dma_start(out=gmB[:], in_=gmp[:].to_broadcast((P, D)))
            btB = cpool.tile([P, D], f32, tag="btB")
            nc.sync.dma_start(out=btB[:], in_=btp[:].to_broadcast((P, D)))

            # AllToAll over all 8 cores: block j (rows 256j..256j+255) goes to
            # core j and holds this core's 4 heads (2 pair-tiles) for q rows
            # [256j, 256j+256) of this core's batch. Core j outputs those q
            # rows for BOTH batches, so every block is meaningful and all
            # offsets are batch-independent (pure SPMD). One collective: its
            # ~40us cost is rendezvous latency, independent of payload size.
            a2a_in = drp.tile([8 * 2 * P, 2 * P], bf16, tag="a2a_in")
            a2a_out = drp.tile([8 * 2 * P, 2 * P], bf16, tag="a2a_out")

            # ---- projections: Q^T, K^T, V^T [GHK, S], all bf16 ----
            qt_sb, kt_sb, vt_sb, va_sb = [], [], [], []
            vt_scope = tc.tile_pool(name="vt", bufs=NT)
            vtp = vt_scope.__enter__()
            with tc.tile_pool(name="xT", bufs=DC) as xtp, \
                 tc.tile_pool(name="w3", bufs=DC) as w3p, \
                 tc.tile_pool(name="pp", bufs=2, space="PSUM") as pp:
                xt_sb, wq_sb, wk_sb, wv_sb = [], [], [], []
                for dc in range(DC):
                    t = w3p.tile([P, GHK], bf16, tag="wq")
                    nc.sync.dma_start(out=t[:], in_=wq[dc * P:(dc + 1) * P, :])
                    wq_sb.append(t)
                # xT loaded in [P, QW] chunks, m-major, so the chunks feeding
                # the first accumulation group land on many DMA queues in
                # parallel and the first matmul can start ~4x sooner.
                for dc in range(DC):
                    xt_sb.append(xtp.tile([P, S], bf16, tag="xt",
                                          name=f"xt{dc}"))
                for m in range(S // QW):
                    for dc in range(DC):
                        nc.sync.dma_start(
                            out=xt_sb[dc][:, m * QW:(m + 1) * QW],
                            in_=xT[dc * P:(dc + 1) * P, m * QW:(m + 1) * QW])
                for dc in range(DC):
                    t = w3p.tile([P, GHK], bf16, tag="wk")
                    nc.sync.dma_start(out=t[:], in_=wk[dc * P:(dc + 1) * P, :])
                    wk_sb.append(t)
                for dc in range(DC):
                    t = w3p.tile([P, GHK], bf16, tag="wv")
                    nc.sync.dma_start(out=t[:], in_=wv[dc * P:(dc + 1) * P, :])
                    wv_sb.append(t)

                def project(w_sb, b_sb, out_list, pool, tag):
                    for t in range(NT):
                        pq = pp.tile([P, S], f32, tag="pp")
                        for m in range(S // QW):
                            for dc in range(DC):
                                nc.tensor.matmul(
                                    pq[:, m * QW:(m + 1) * QW],
                                    w_sb[dc][:, t * P:(t + 1) * P],
                                    xt_sb[dc][:, m * QW:(m + 1) * QW],
                                    start=(dc == 0), stop=(dc == DC - 1))
                        ot = pool.tile([P, S], bf16, tag=tag)
                        nc.vector.tensor_scalar(ot[:], pq[:], b_sb[:, t:t + 1],
                                                None, OP.add)
                        out_list.append(ot)

                project(wq_sb, bq_sb, qt_sb, qkp, "qt")
                project(wk_sb, bk_sb, kt_sb, qkp, "kt")
                project(wv_sb, bv_sb, vt_sb, vtp, "vt")
            # ---- V to natural layout [s, hk] via PE transpose ----
            with tc.tile_pool(name="tp", bufs=4, space="PSUM") as tpp:
                for sc in range(NSC):
                    pvt = tpp.tile([P, GHK], bf16, tag="tp")
                    with nc.allow_low_precision(
                            reason="pure transpose, no accumulation"):
                        for t in range(NT):
                            nc.tensor.transpose(
                                pvt[:, t * P:(t + 1) * P],
                                vt_sb[t][:, sc * P:(sc + 1) * P],
                                ident[:])
                    va = vap.tile([P, GHK], bf16, tag="va")
                    nc.vector.tensor_copy(va[:], pvt[:])
                    va_sb.append(va)
            vt_scope.__exit__(None, None, None)

            # wo / xq loads (needed late; queue after projection loads)
            wo_sb = []
            for hc in range(HC):
                t = wop.tile([P, D], bf16, tag="wo")
                nc.sync.dma_start(out=t[:], in_=wo[hc * P:(hc + 1) * P, :])
                wo_sb.append(t)
            xq_sb = []
            with tc.tile_pool(name="xq", bufs=4) as xqp:
                for qs in range(SB // P):
                    t = xqp.tile([P, D], f32, tag="xq")
                    nc.sync.dma_start(out=t[:], in_=xqb[qs * P:(qs + 1) * P, :])
                    xq_sb.append(t)

                # ---- attention, qc-major: per sc both pairs' exps run on ----
                # ---- both exp engines concurrently (strict alternation) ----
                QA = QW
                with tc.tile_pool(name="ps", bufs=2, space="PSUM") as psp, \
                     tc.tile_pool(name="pc", bufs=3, space="PSUM") as pcp, \
                     tc.tile_pool(name="dn", bufs=1, space="PSUM") as dnp, \
                     tc.tile_pool(name="et", bufs=6) as etp, \
                     tc.tile_pool(name="dsb", bufs=2) as dsbp, \
                     tc.tile_pool(name="rb", bufs=2) as rbp:
                    for qc in range(S // QA):
                        pc = [pcp.tile([P, QA], f32, tag="pc",
                                       name=f"pc{qc}_{i}") for i in range(NT)]
                        pdn = dnp.tile([P, QA], f32, tag="dn",
                                       name=f"pdn{qc}")
                        nc.vector.memset(pdn[:], 0.0)
                        ets = [None] * NSC

                        def emit_scores(sc):
                            pair_ets = []
                            for pr in range(NT):
                                ps = psp.tile([P, 2 * QA], f32, tag="ps")
                                for j in range(2):
                                    nc.tensor.matmul(
                                        ps[:, j * QA:(j + 1) * QA],
                                        kt_sb[pr][j * KD:(j + 1) * KD,
                                                  sc * P:(sc + 1) * P],
                                        qt_sb[pr][j * KD:(j + 1) * KD,
                                                  qc * QA:(qc + 1) * QA],
                                        start=True, stop=True)
                                # one full-width exp per pair tile; pairs
                                # alternate strictly between ACT (exact exp)
                                # and DVE (Schraudolph bitcast exp) so both
                                # engines run every sc step
                                if (pr + sc) % 2 == 0:
                                    et = etp.tile([P, 2 * QA], bf16, tag="et")
                                    nc.scalar.activation(et[:], ps[:], AF.Exp,
                                                         scale=1.0 / EXPA)
                                    etv = et[:]
                                else:
                                    et = etp.tile([P, 2 * QA], i16, tag="et")
                                    nc.vector.tensor_scalar(
                                        et[:], ps[:], EXPB, 0.0,
                                        OP.add, OP.max)
                                    etv = et[:].bitcast(bf16)
                                pair_ets.append(etv)
                            ets[sc] = pair_ets

                        def emit_ctx(sc):
                            for pr in range(NT):
                                etv = ets[sc][pr]
                                for j in range(2):
                                    nc.tensor.matmul(
                                        pc[pr][j * KD:(j + 1) * KD, :],
                                        va_sb[sc][:, (2 * pr + j) * KD:
                                                  (2 * pr + j + 1) * KD],
                                        etv[:, j * QA:(j + 1) * QA],
                                        start=(sc == 0), stop=(sc == NSC - 1))
                            for pr in range(NT):
                                etv = ets[sc][pr]
                                for j in range(2):
                                    r = 32 * (2 * pr + j)
                                    nc.tensor.matmul(
                                        pdn[r:r + 1, :],
                                        ones_bf[:],
                                        etv[:, j * QA:(j + 1) * QA],
                                        start=(sc == 0), stop=(sc == NSC - 1),
                                        tile_position=(0, r))
                            ets[sc] = None

                        for sc in range(NSC):
                            emit_scores(sc)
                            if sc > 0:
                                emit_ctx(sc - 1)
                        emit_ctx(NSC - 1)

                        # normalize: cn = pc * (1/denom bcast); the
                        # reciprocal runs as exp(-ln(x)) on ACT (the DVE
                        # reciprocal instruction is ~4 cycles/element)
                        dn_t = dsbp.tile([P, QA], bf16, tag="dsb")
                        nc.vector.tensor_copy(dn_t[:], pdn[:])
                        for pr in range(NT):
                            prb = dnp.tile([P, QA], f32, tag="dn",
                                           name=f"prb{qc}_{pr}")
                            nc.tensor.matmul(prb[:],
                                             selm[:, pr * P:(pr + 1) * P],
                                             dn_t[:], start=True, stop=True)
                            lnt = rbp.tile([P, QA], f32, tag="lnt")
                            nc.scalar.activation(lnt[:], prb[:], AF.Ln)
                            rb_t = rbp.tile([P, QA], bf16, tag="rb")
                            nc.scalar.activation(rb_t[:], lnt[:], AF.Exp,
                                                 scale=-1.0)
                            cn = cnp.tile([P, QA], bf16, tag="cn")
                            nc.vector.tensor_tensor(cn[:], pc[pr][:], rb_t[:],
                                                    OP.mult)
                            for h in range(2):
                                blk = 2 * qc + h
                                nc.sync.dma_start(
                                    out=a2a_in[blk * 2 * P + pr * P:
                                               blk * 2 * P + (pr + 1) * P, :],
                                    in_=cn[:, h * 2 * P:(h + 1) * 2 * P])

                # ---- AllToAll across all 8 cores ----
                nc.gpsimd.collective_compute(
                    "AllToAll", mybir.AluOpType.bypass,
                    replica_groups=[[0, 1, 2, 3, 4, 5, 6, 7]],
                    ins=[a2a_in[:].opt()],
                    outs=[a2a_out[:].opt()])

                # ---- output projection + residual + LayerNorm ----
                with tc.tile_pool(name="ctxa", bufs=2 * HC) as ctxp, \
                     tc.tile_pool(name="op", bufs=2, space="PSUM") as opp, \
                     tc.tile_pool(name="ln", bufs=2) as lnp, \
                     tc.tile_pool(name="st", bufs=4) as stp:
                    ctx_all = {}
                    for b_ in range(2):
                        for hc in range(HC):
                            t = ctxp.tile([P, 2 * P], bf16, tag="ctxa")
                            src = (4 * b_ + hc // 2) * 2 * P + (hc % 2) * P
                            nc.sync.dma_start(
                                out=t[:], in_=a2a_out[src:src + P, :])
                            ctx_all[(b_, hc)] = t
                    for qs in range(SB // P):
                        b_, q2 = qs // 2, qs % 2
                        po = opp.tile([P, D], f32, tag="op")
                        for d5 in range(D // QW):
                            for hc in range(HC):
                                nc.tensor.matmul(
                                    po[:, d5 * QW:(d5 + 1) * QW],
                                    ctx_all[(b_, hc)][:, q2 * P:(q2 + 1) * P],
                                    wo_sb[hc][:, d5 * QW:(d5 + 1) * QW],
                                    start=(hc == 0), stop=(hc == HC - 1))
                        y = lnp.tile([P, D], bf16, tag="y")
                        nc.vector.tensor_tensor(y[:], po[:], xq_sb[qs][:],
                                                OP.add)
                        sum_t = stp.tile([P, 1], f32, tag="sum")
                        nc.vector.reduce_sum(out=sum_t[:], in_=y[:], axis=AX)
                        mean_t = stp.tile([P, 1], f32, tag="mean")
                        nc.vector.tensor_scalar_mul(mean_t[:], sum_t[:],
                                                    1.0 / D)
                        cent = lnp.tile([P, D], bf16, tag="cent")
                        nc.vector.tensor_scalar(cent[:], y[:], mean_t[:],
                                                None, OP.subtract)
                        sq = lnp.tile([P, D], bf16, tag="sq")
                        vs = stp.tile([P, 1], f32, tag="vs")
                        nc.scalar.activation(sq[:], cent[:], AF.Square,
                                             accum_out=vs[:])
                        std = stp.tile([P, 1], f32, tag="std")
                        nc.scalar.activation(std[:], vs[:], AF.Sqrt,
                                             bias=eps_t[:], scale=1.0 / D)
                        rstd = stp.tile([P, 1], f32, tag="rstd")
                        nc.vector.reciprocal(rstd[:], std[:])
                        z = lnp.tile([P, D], bf16, tag="z")
                        nc.vector.tensor_scalar_mul(z[:], cent[:], rstd[:])
                        zg = lnp.tile([P, D], bf16, tag="zg")
                        nc.vector.tensor_tensor(zg[:], z[:], gmB[:], OP.mult)
                        ot = lnp.tile([P, D], f32, tag="ot")
                        nc.vector.tensor_tensor(ot[:], zg[:], btB[:], OP.add)
                        nc.sync.dma_start(out=out[qs * P:(qs + 1) * P, :],
                                          in_=ot[:])

    # Post-pass: walrus's per-instruction ISA structs hold only ONE sync
    # wait for compute-engine instructions. Move excess waits onto standalone
    # EventSemaphore instructions placed just before on the same engine
    # stream (sequencer executes them in order; semantics unchanged).
    SPLIT = {"InstMatmult", "InstTensorScalarPtr", "InstTensorScalar",
             "InstTensorTensor", "InstReciprocal", "InstActivation",
             "InstTensorReduce", "InstTensorCopy", "InstMemSet",
             "InstCopy", "InstDMACopy", "InstDMATranspose", "InstDrain",
             "InstCollectiveCompute", "InstMemset"}
    evt_n = 0
    for f in nc.m.functions:
        for bb in f.blocks:
            need = any(
                type(i).__name__ in SPLIT and i.sync_info is not None
                and len(i.sync_info.on_wait) > 1 for i in bb.instructions)
            if not need:
                continue
            newl = []
            for ins in bb.instructions:
                si = ins.sync_info
                if (type(ins).__name__ in SPLIT and si is not None
                        and len(si.on_wait) > 1):
                    extra = list(si.on_wait[:-1])
                    for j in range(0, len(extra), 2):  # evt-sem holds <=2
                        evt_n += 1
                        evt = mybir.InstEventSemaphore(name=f"mmwait_{evt_n}")
                        evt.engine = ins.engine
                        evt.sync_info = mybir.SyncInfo(
                            on_wait=extra[j:j + 2], on_update=[])
                        newl.append(evt)
                    ins.sync_info = mybir.SyncInfo(
                        on_wait=[si.on_wait[-1]],
                        on_update=list(si.on_update))
                newl.append(ins)
            bb.instructions = newl
    return nc


def get_nc():
    if "nc" not in _cache:
        _cache["nc"] = _build()
    return _cache["nc"]


def make_in_maps(inputs, n_cores=8):
    """Shard full inputs into per-core input maps (host-side layout prep)."""
    from ml_dtypes import bfloat16
    f = np.float32
    HK = H * KD
    qscale = EXPA / np.sqrt(KD)
    wq_ = np.asarray(inputs["wq"], f).reshape(D, HK) * qscale
    wk_ = np.asarray(inputs["wk"], f).reshape(D, HK)
    wv_ = np.asarray(inputs["wv"], f).reshape(D, HK)
    wo_ = np.ascontiguousarray(
        np.asarray(inputs["wo"], f).reshape(HK, D)).astype(bfloat16)
    bq_ = np.asarray(inputs["bq"], f).reshape(HK) * qscale
    bk_ = np.asarray(inputs["bk"], f).reshape(HK)
    bv_ = np.asarray(inputs["bv"], f).reshape(HK)
    bo_ = np.asarray(inputs["bo"], f).reshape(D)
    gm_row = np.asarray(inputs["gamma"], f).reshape(1, D).astype(bfloat16)
    bt_row = np.asarray(inputs["beta"], f).reshape(1, D)
    ident = np.eye(P, dtype=f).astype(bfloat16)
    selm = np.zeros((P, 2 * P), f)
    for pr in range(NT):
        selm[64 * pr, pr * P: pr * P + 64] = 1.0
        selm[64 * pr + 32, pr * P + 64: pr * P + 128] = 1.0
    selm = selm.astype(bfloat16)

    xT_cache, x_cache = {}, {}
    wcache = {}
    maps = []
    for c in range(n_cores):
        b, g = c // 4, c % 4
        if b not in xT_cache:
            xb = np.asarray(inputs["x"][b], f)
            x_cache[b] = xb
            xT_cache[b] = np.ascontiguousarray(xb.T).astype(bfloat16)
        if g not in wcache:
            hk = slice(g * GHK, (g + 1) * GHK)
            wcache[g] = dict(
                wq=np.ascontiguousarray(wq_[:, hk]).astype(bfloat16),
                wk=np.ascontiguousarray(wk_[:, hk]).astype(bfloat16),
                wv=np.ascontiguousarray(wv_[:, hk]).astype(bfloat16),
                bqT=np.ascontiguousarray(bq_[hk].reshape(NT, P).T),
                bkT=np.ascontiguousarray(bk_[hk].reshape(NT, P).T),
                bvT=np.ascontiguousarray(bv_[hk].reshape(NT, P).T),
            )
        # output rows [256c, 256c+256) of BOTH batches (residual+bias input)
        xqb = np.concatenate([
            x_cache.setdefault(
                bb, np.asarray(inputs["x"][bb], f))[
                    256 * c:256 * (c + 1)] + bo_
            for bb in range(2)], axis=0)
        maps.append(dict(
            xT=xT_cache[b],
            wo=wo_,
            xqb=xqb,
            gamma_row=gm_row, beta_row=bt_row,
            ident=ident, selm=selm,
            **wcache[g],
        ))
    return maps


def assemble(res):
    outp = np.empty((B, S, D), np.float32)
    for c in range(8):
        o = res.results[c]["out"]
        for b_ in range(2):
            outp[b_, 256 * c:256 * (c + 1)] = o[256 * b_:256 * (b_ + 1)]
    return outp


def kernel(**inputs):
    from concourse.bass_utils import run_bass_kernel_spmd
    nc = get_nc()
    maps = make_in_maps(inputs)
    res = run_bass_kernel_spmd(nc, maps, list(range(8)))
    return assemble(res)
